# revision 12
# baseline (speedup 1.0000x reference)
"""BiMamba block kernel for 8 Trainium2 NeuronCores.

Sharding: core = 2*sample + direction (4 samples x 2 scan directions).
Each core runs the full mamba for its (sample, direction).

v2 layout: the selective scan dominates on DVE (16 states x 4096 cols at
~2.1 ns/col is irreducible), so the kernel is organized to keep the DVE
queue dense and hide everything else under it:

- Front: fused in-proj + causal depthwise conv as 4 accumulating
  matmuls; true-Silu activations (chunks 0-3 via the silu table before
  any exp work; chunks 4-7 via the tanh identity inside the exp table
  set so no table reload interrupts the first segment's exps).
- Scan: 3 segments (2048/1536/512 cols).  Per state: ACT exp ->
  DVE dbx mul -> DVE tensor_tensor_scan -> DVE C mul -> PE PSUM
  accumulate.  B/C rows arrive via DRAM partition-broadcast DMAs,
  issued a few states ahead.
- Tail: after each segment the out-projection halves are exchanged with
  the pair core via a masked AllReduce (own direction's rows scaled by
  a 0/1 mask from the blob, so the reduce concatenates), giving every
  core the full 128-channel field; the 3x3 conv then runs locally with
  no collective after it.  The reference never un-flips y2, so the odd
  core's natural order already matches the conv row order.  BatchNorm
  stats AllReduce over all 8 cores (each sample counted twice); invstd
  via ln/exp so no extra ACT table load on the critical path.
"""
import os
import sys

for _p in ("/opt/trn_rl_repo", "/root/.axon_site/_ro/trn_rl_repo"):
    if os.path.isdir(_p):
        if _p not in sys.path:
            sys.path.insert(0, _p)
        break

import ml_dtypes
import numpy as np

# The agent image's antenv lacks axon_hooks; inject it so trace=True can
# capture NTFF profiles (used by test.py for HW timing, not for grading).
try:
    import antenv.axon_hooks  # noqa: F401
except ImportError:
    try:
        import types as _types

        from trn_agent_boot.trn_boot import _ntff_profile_via_ctypes

        _hook = _ntff_profile_via_ctypes("/opt/axon/libaxon_pjrt.so")
        _m = _types.ModuleType("antenv.axon_hooks")
        _m.get_axon_ntff_profile_hook = lambda: _hook
        _m.set_axon_ntff_profile_hook = lambda h: None
        sys.modules["antenv.axon_hooks"] = _m
    except Exception:
        pass

import concourse.bass as bass
import concourse.mybir as mybir
from concourse import bacc
from concourse import bass_utils
from concourse.masks import make_identity
from concourse.tile import TileContext

F32 = mybir.dt.float32
BF16 = mybir.dt.bfloat16
AF = mybir.ActivationFunctionType
OP = mybir.AluOpType

B, C, H, W = 4, 64, 64, 64
L = H * W          # 4096
DI = 128           # d_inner
DS = 16            # d_state
DTR = 4            # dt_rank
DCONV = 4
NCORE = 8
CH = 512           # matmul free-dim chunk
NCH = L // CH      # 8
RPC = CH // W      # output rows per chunk (8)

SEGS = ((0, 2048), (2048, 3584), (3584, 4096))
SEG_CHUNKS = ((0, 1, 2, 3), (4, 5, 6), (7,))
NSEG = len(SEGS)

BH_COLS = 9 * C + C + 128 + 32   # c3w | owT | bigT | bcwT
BF_COLS = 736


def _build():
    nc = bacc.Bacc(target_bir_lowering=False, debug=False, num_devices=NCORE)

    def din(name, shape, dtype=F32):
        return nc.dram_tensor(name, shape, dtype, kind="ExternalInput")

    F32R = mybir.dt.float32r
    x_loc = din("x_loc", [C, L], F32R)
    blob_f = din("blob_f", [128, BF_COLS], F32R)
    blob_h = din("blob_h", [128, BH_COLS], BF16)

    out_d = nc.dram_tensor("out", [C, L], F32, kind="ExternalOutput")

    with TileContext(nc) as tc:
        with tc.tile_pool(name="pers", bufs=1) as pers:
            # ---- params arrive as two packed blobs ----
            p_bf = pers.tile([128, BF_COLS], F32R)
            p_bh = pers.tile([128, BH_COLS], BF16)
            nc.sync.dma_start(p_bf[:], blob_f[:])
            nc.sync.dma_start(p_bh[:], blob_h[:])
            p_wk = [p_bf[:, 128 * k:128 * (k + 1)] for k in range(DCONV)]
            p_zwT = p_bf[:, 512:640]
            p_rwT = p_bf[:, 640:704]
            p_c1b = p_bf[:, 704:705].bitcast(F32)
            p_dtb = p_bf[:, 705:706].bitcast(F32)
            p_A = p_bf[:, 706:722].bitcast(F32)
            p_D = p_bf[:, 722:723].bitcast(F32)
            p_c3b = p_bf[:C, 723:724].bitcast(F32)
            p_rb = p_bf[:C, 724:725].bitcast(F32)
            p_bng = p_bf[:C, 725:726].bitcast(F32)
            p_bnb = p_bf[:C, 726:727].bitcast(F32)
            p_mask = p_bf[:, 727:728].bitcast(F32)   # [128,1] direction mask
            p_c1bh = p_bf[:, 728:729].bitcast(F32)   # 0.5*conv1 bias
            p_c3w = p_bh[:, 0:9 * C]
            p_owT = p_bh[:, 9 * C:9 * C + C]
            p_bigT = p_bh[:, 9 * C + C:9 * C + C + 128]
            p_bcwT = p_bh[:, 9 * C + C + 128:9 * C + C + 160]

            ident = pers.tile([128, 128], F32)
            make_identity(nc, ident[:])
            ident_g = pers.tile([128, 128], BF16)
            nc.vector.tensor_copy(ident_g[:], ident[:])

            # DRAM staging for B/C rows (DMA partition-broadcast needs a
            # DRAM source)
            bc_dram = nc.dram_tensor("bc_stage", [2 * DS, L], BF16)

            x_pad = pers.tile([64, 3 + L], F32R)
            nc.gpsimd.memset(x_pad[:, 0:3].bitcast(F32), 0.0)
            # split load so chunk-0 matmuls start early
            nc.sync.dma_start(x_pad[:, 3:3 + 1024], x_loc[:, 0:1024])
            nc.sync.dma_start(x_pad[:, 3 + 1024:3 + L], x_loc[:, 1024:L])

            with tc.tile_pool(name="smid", bufs=1) as smid, \
                 tc.tile_pool(name="ps", bufs=4, space="PSUM") as psp, \
                 tc.tile_pool(name="psy", bufs=4, space="PSUM") as psy, \
                 tc.tile_pool(name="sl_e", bufs=2) as plex, \
                 tc.tile_pool(name="sl_a", bufs=3) as pla, \
                 tc.tile_pool(name="sl_b", bufs=4) as plb, \
                 tc.tile_pool(name="sl_x", bufs=3) as plx, \
                 tc.tile_pool(name="sl_h", bufs=3) as plh, \
                 tc.tile_pool(name="sl_c", bufs=4) as plc, \
                 tc.tile_pool(name="sl_g", bufs=3) as plg, \
                 tc.tile_pool(name="sl_f", bufs=2) as plf, \
                 tc.tile_pool(name="dram", bufs=1, space="DRAM") as dr:
                z_sil = smid.tile([DI, L], BF16)
                dtv = smid.tile([DI, L], BF16)
                dtxc = smid.tile([DI, L], BF16)
                xcd = smid.tile([DI, L], BF16)
                xc = smid.tile([DI, L], BF16)
                carry = smid.tile([DI, DS], F32)

                ympad = smid.tile([128, H + 2, W + 2], BF16)
                nc.gpsimd.memset(ympad[:], 0.0)
                res_sb = smid.tile([C, L], BF16)
                conv_sb = smid.tile([C, L], BF16)
                stats_m = smid.tile([C, NCH], F32)
                stats_v = smid.tile([C, NCH], F32)
                PAIRS = [[0, 1], [2, 3], [4, 5], [6, 7]]

                cc_ins, cc_outs = [], []
                for wi, cvs in enumerate(SEG_CHUNKS):
                    cc_ins.append(dr.tile([128, len(cvs) * CH], BF16,
                                          name=f"cci{wi}"))
                    cc_outs.append(dr.tile([128, len(cvs) * CH], BF16,
                                           name=f"cco{wi}"))
                st_in = dr.tile([C, 2], F32)
                st_out = nc.dram_tensor("st_out", [C, 2], F32,
                                        addr_space="Shared")

                def in_proj(c):
                    """Fused in-proj + causal depthwise conv and gate
                    projection for chunk c; returns the two PSUM tiles."""
                    ps = psp.tile([128, CH], F32, tag="ps")
                    for k in range(DCONV):
                        nc.tensor.matmul(ps[:DI], p_wk[k][:C],
                                         x_pad[:, c * CH + k:c * CH + k + CH],
                                         start=(k == 0), stop=(k == DCONV - 1))
                    ps2 = psp.tile([128, CH], F32, tag="ps")
                    nc.tensor.matmul(ps2[:DI], p_zwT[:C],
                                     x_pad[:, 3 + c * CH:3 + (c + 1) * CH],
                                     start=True, stop=True)
                    return ps, ps2

                def silu_direct(c, ps, ps2):
                    # true Silu from the silu table set (front chunks 0-3)
                    sl = slice(c * CH, (c + 1) * CH)
                    nc.scalar.activation(xc[:, sl], ps[:DI], AF.Silu,
                                         bias=p_c1b[:, 0:1])
                    nc.scalar.activation(z_sil[:, sl], ps2[:DI], AF.Silu)

                def silu_tanh(c, ps, ps2):
                    # silu via 0.5*(v+b)*(1+tanh((v+b)/2)) -- tanh lives in
                    # the exp table set, so chunks 4-7 can run between the
                    # segment-0 exps without a table reload
                    sl = slice(c * CH, (c + 1) * CH)
                    for src, dst, hb in ((ps, xc, p_c1bh), (ps2, z_sil, None)):
                        kw = {"bias": hb[:, 0:1]} if hb is not None else {}
                        th = plf.tile([DI, CH], BF16, tag="th")
                        nc.scalar.activation(th[:], src[:DI], AF.Tanh,
                                             scale=0.5, **kw)
                        raw = plf.tile([DI, CH], BF16, tag="raw")
                        nc.scalar.activation(raw[:], src[:DI], AF.Identity,
                                             scale=0.5, **kw)
                        nc.vector.scalar_tensor_tensor(
                            dst[:, sl], th[:], 1.0, raw[:],
                            op0=OP.add, op1=OP.mult)

                def dt_bc(c):
                    """dt projection (exp part) + B/C projection for chunk
                    c.  Softplus is finished by a batched Ln later."""
                    sl = slice(c * CH, (c + 1) * CH)
                    ps3 = psp.tile([128, CH], F32, tag="ps")
                    nc.tensor.matmul(ps3[:DI], p_bigT[:], xc[:, sl],
                                     start=True, stop=True)
                    nc.scalar.activation(dtv[:, sl], ps3[:DI], AF.Exp,
                                         bias=p_dtb[:, 0:1])
                    ps4 = psp.tile([128, CH], F32, tag="ps")
                    nc.tensor.matmul(ps4[:2 * DS], p_bcwT[:], xc[:, sl],
                                     start=True, stop=True)
                    bch = plb.tile([2 * DS, CH], BF16, tag="bch")
                    nc.scalar.copy(bch[:], ps4[:2 * DS])
                    nc.sync.dma_start(bc_dram[:, sl], bch[:])

                def post_softplus(cs):
                    hsl = slice(cs[0] * CH, (cs[-1] + 1) * CH)
                    nc.scalar.activation(dtv[:, hsl], dtv[:, hsl], AF.Ln,
                                         bias=1.0)
                    for c in cs:
                        sl = slice(c * CH, (c + 1) * CH)
                        nc.vector.tensor_mul(dtxc[:, sl], dtv[:, sl],
                                             xc[:, sl])
                        nc.scalar.activation(xcd[:, sl], xc[:, sl],
                                             AF.Copy, scale=p_D[:, 0:1])

                def seg_end(q):
                    """Out-projection, exchange staging + AllReduce, ympad
                    write and residual for segment q's chunks."""
                    cvs = SEG_CHUNKS[q]
                    stage = plex.tile([128, len(cvs) * CH], BF16,
                                      tag="stage", name=f"stage{q}")
                    for j, cix in enumerate(cvs):
                        sl = slice(cix * CH, (cix + 1) * CH)
                        ssl = slice(j * CH, (j + 1) * CH)
                        yg = plf.tile([DI, CH], BF16, tag="yg")
                        nc.vector.tensor_mul(yg[:], y_ps[cix][:DI],
                                             z_sil[:, sl])
                        po = psp.tile([128, CH], F32, tag="ps",
                                      name=f"po{cix}")
                        nc.tensor.matmul(po[:C], p_owT[:], yg[:],
                                         start=True, stop=True)
                        # masked staging: own direction's rows keep po, the
                        # other direction's rows are zeroed; the pair
                        # AllReduce then concatenates the two halves
                        nc.scalar.activation(stage[0:C, ssl], po[:C],
                                             AF.Copy, scale=p_mask[0:C, 0:1])
                        nc.scalar.activation(stage[C:128, ssl], po[:C],
                                             AF.Copy, scale=p_mask[C:128, 0:1])
                        psr = psp.tile([128, CH], F32, tag="ps",
                                       name=f"rs{cix}")
                        nc.tensor.matmul(psr[:C], p_rwT[:C],
                                         x_pad[:, 3 + cix * CH:
                                               3 + (cix + 1) * CH],
                                         start=True, stop=True)
                        nc.scalar.activation(res_sb[:, sl], psr[:C],
                                             AF.Identity, bias=p_rb[:, 0:1])
                    nc.sync.dma_start(cc_ins[q][:], stage[:])
                    nc.gpsimd.collective_compute(
                        "AllReduce", OP.add, replica_groups=PAIRS,
                        ins=[cc_ins[q][:].opt()], outs=[cc_outs[q][:].opt()])
                    r0 = cvs[0] * RPC
                    nrows = len(cvs) * RPC
                    nc.sync.dma_start(
                        ympad[:, 1 + r0:1 + r0 + nrows, 1:1 + W],
                        cc_outs[q][:].rearrange("p (r w) -> p r w", w=W))

                def conv3_chunk(c):
                    ps = psp.tile([128, CH], F32, tag="ps", name=f"cv{c}")
                    ps3 = ps[:C].rearrange("p (r w) -> p r w", w=W)
                    r0 = c * RPC
                    n = 0
                    for ky in range(3):
                        for kx in range(3):
                            nc.tensor.matmul(
                                ps3[:],
                                p_c3w[:, (ky * 3 + kx) * C:
                                      (ky * 3 + kx + 1) * C],
                                ympad[:, r0 + ky:r0 + ky + RPC, kx:kx + W],
                                start=(n == 0), stop=(n == 8))
                            n += 1
                    sl = slice(c * CH, (c + 1) * CH)
                    flat = ps3.rearrange("p r w -> p (r w)")
                    nc.scalar.activation(conv_sb[:, sl], flat,
                                         AF.Identity, bias=p_c3b[:, 0:1],
                                         accum_out=stats_m[:, c:c + 1])
                    sq = plf.tile([C, CH], BF16, tag="sq")
                    nc.scalar.activation(sq[:], conv_sb[:, sl],
                                         AF.Square,
                                         accum_out=stats_v[:, c:c + 1])

                # =========== emission ===========
                # front chunks 0-3: silu-table silus first
                for c in range(4):
                    ps, ps2 = in_proj(c)
                    silu_direct(c, ps, ps2)
                # nl_exp set: dt exp + softplus + staging for chunks 0-3
                for c in range(4):
                    dt_bc(c)
                post_softplus((0, 1, 2, 3))

                y_ps = {}

                def seg_scan(q, front_work):
                    t0, t1 = SEGS[q]
                    SEG = t1 - t0
                    qsl = slice(t0, t1)
                    for cix in SEG_CHUNKS[q]:
                        yp = psy.tile([128, CH], F32, tag="yps",
                                      name=f"y{cix}")
                        nc.tensor.matmul(yp[:DI], ident_g[:],
                                         xcd[:, cix * CH:(cix + 1) * CH],
                                         start=True, stop=False)
                        y_ps[cix] = yp
                    for s in range(DS):
                        da = pla.tile([DI, SEG], BF16, tag="da")
                        nc.scalar.activation(da[:], dtv[:, qsl], AF.Exp,
                                             scale=p_A[:, s:s + 1])
                        bbc = plb.tile([DI, SEG], BF16, tag="bbc")
                        nc.sync.dma_start(
                            bbc[:],
                            bc_dram[s:s + 1, qsl].to_broadcast((DI, SEG)))
                        dbx = plx.tile([DI, SEG], BF16, tag="dbx")
                        nc.vector.tensor_mul(dbx[:], dtxc[:, qsl], bbc[:])
                        h = plh.tile([DI, SEG], BF16, tag="h")
                        init = 0.0 if q == 0 else carry[:, s:s + 1]
                        nc.vector.tensor_tensor_scan(h[:], da[:], dbx[:],
                                                     init, op0=OP.mult,
                                                     op1=OP.add)
                        if q < NSEG - 1:
                            # on DVE so the in-order ACT queue of exps is
                            # never blocked behind a scan result
                            nc.vector.tensor_copy(carry[:, s:s + 1],
                                                  h[:, SEG - 1:SEG])
                        cbc = plc.tile([DI, SEG], BF16, tag="cbc")
                        nc.sync.dma_start(
                            cbc[:],
                            bc_dram[DS + s:DS + s + 1, qsl].to_broadcast(
                                (DI, SEG)))
                        g = plg.tile([DI, SEG], BF16, tag="g")
                        nc.vector.tensor_mul(g[:], h[:], cbc[:])
                        for j, cix in enumerate(SEG_CHUNKS[q]):
                            nc.tensor.matmul(
                                y_ps[cix][:DI], ident_g[:],
                                g[:, j * CH:(j + 1) * CH],
                                start=False, stop=(s == DS - 1))
                        if s in front_work:
                            front_work[s]()

                # segment 0 with front chunks 4-7 interleaved between
                # states (tanh-form silu: stays inside the exp table set)
                def fw(c, with_ln):
                    def go():
                        ps, ps2 = in_proj(c)
                        silu_tanh(c, ps, ps2)
                        dt_bc(c)
                        if with_ln:
                            post_softplus((4, 5, 6, 7))
                    return go

                seg_scan(0, {2: fw(4, False), 5: fw(5, False),
                             8: fw(6, False), 11: fw(7, True)})
                seg_end(0)
                seg_scan(1, {3: lambda: conv3_chunk(0),
                             7: lambda: conv3_chunk(1),
                             11: lambda: conv3_chunk(2)})
                seg_end(1)
                seg_scan(2, {4: lambda: conv3_chunk(3),
                             8: lambda: conv3_chunk(4),
                             12: lambda: conv3_chunk(5)})
                seg_end(2)
                conv3_chunk(6)
                conv3_chunk(7)

                # ---- batch stats AllReduce + BN + residual + leaky ----
                tl = smid
                stats = tl.tile([C, 2], F32)
                nc.vector.tensor_reduce(stats[:, 0:1], stats_m[:],
                                        axis=mybir.AxisListType.X, op=OP.add)
                nc.vector.tensor_reduce(stats[:, 1:2], stats_v[:],
                                        axis=mybir.AxisListType.X, op=OP.add)
                nc.sync.dma_start(st_in[:], stats[:])
                nc.gpsimd.collective_compute(
                    "AllReduce", OP.add,
                    replica_groups=[[0, 1, 2, 3, 4, 5, 6, 7]],
                    ins=[st_in[:].opt()], outs=[st_out[:].opt()])
                stot = tl.tile([C, 2], F32)
                nc.sync.dma_start(stot[:], st_out[:])

                # every sample's full conv is present on both pair cores,
                # so the 8-core sum double counts: divide by 2*B*L
                inv = 1.0 / (2.0 * B * L)
                mean = tl.tile([C, 1], F32)
                ex2 = tl.tile([C, 1], F32)
                var = tl.tile([C, 1], F32)
                tmp = tl.tile([C, 1], F32)
                nc.vector.tensor_scalar_mul(mean[:], stot[:, 0:1], inv)
                nc.vector.tensor_scalar_mul(ex2[:], stot[:, 1:2], inv)
                nc.vector.tensor_mul(tmp[:], mean[:], mean[:])
                nc.vector.tensor_sub(var[:], ex2[:], tmp[:])
                # invstd = exp(-0.5*ln(var+eps)) -- ln/exp stay in the
                # loaded table set (no sqrt-set reload on the tail)
                nc.vector.tensor_scalar_add(var[:], var[:], 1e-5)
                nc.scalar.activation(tmp[:], var[:], AF.Ln)
                nc.scalar.activation(tmp[:], tmp[:], AF.Exp, scale=-0.5)
                scal = tl.tile([C, 1], F32)
                shft = tl.tile([C, 1], F32)
                nc.vector.tensor_mul(scal[:], p_bng[:], tmp[:])
                nc.vector.tensor_mul(tmp[:], mean[:], scal[:])
                nc.vector.tensor_sub(shft[:], p_bnb[:], tmp[:])

                # bn + residual + leaky relu: out = prelu(conv*scal + res
                # + shft); conv*scal on ACT (per-partition scale), add on
                # DVE at 2x, prelu+shift on ACT straight to f32 out
                for lo in range(0, L, 1024):
                    hi = lo + 1024
                    bs = plf.tile([C, 1024], BF16, tag="bn")
                    nc.scalar.activation(bs[:], conv_sb[:, lo:hi],
                                         AF.Copy, scale=scal[:, 0:1])
                    nc.vector.tensor_add(bs[:], bs[:], res_sb[:, lo:hi])
                    ot = plf.tile([C, 1024], F32, tag="ot")
                    nc.scalar.activation(ot[:], bs[:],
                                         AF.Prelu, alpha=0.01,
                                         bias=shft[:, 0:1])
                    nc.sync.dma_start(out_d[:, lo:hi], ot[:])

    nc.compile()
    return nc


_NC = None


def _get_nc():
    global _NC
    if _NC is None:
        _NC = _build()
    return _NC


def _prep_in_maps(inp):
    inp = {k: np.asarray(v, dtype=np.float32) for k, v in inp.items()}
    x = inp["x"]  # (4, 64, 64, 64)
    # full 3x3 conv weights over both direction blocks, [in=128, 9*64]
    c3 = np.zeros((128, 9 * C), np.float32)
    for ky in range(3):
        for kx in range(3):
            c3[:, (ky * 3 + kx) * C:(ky * 3 + kx + 1) * C] = \
                inp["conv_w"][:, :, ky, kx].T
    maps = []
    for core in range(NCORE):
        b, d = core // 2, core % 2
        pre = "m1_" if d == 0 else "m2_"
        in_w = inp[pre + "in_w"]          # (256, 64)
        xproj_w = inp[pre + "xproj_w"]    # (36, 128)
        dt_w = inp[pre + "dt_w"]          # (128, 4)
        conv1_w = inp[pre + "conv_w"]     # (128, 4)

        x_loc = x[b].reshape(C, L)
        if d == 1:
            x_loc = x_loc[:, ::-1]

        bigproj = dt_w @ xproj_w[:DTR]    # (128, 128)

        blob_f = np.zeros((128, BF_COLS), np.float32)
        # fused in-projection + depthwise causal conv:
        # W_k[ch_x, di] = in_w[di, ch_x] * conv1_w[di, k]
        xi_w = in_w[:DI]                  # (128, 64)
        for k in range(DCONV):
            blob_f[:C, 128 * k:128 * (k + 1)] = \
                (xi_w * conv1_w[:, k:k + 1]).T
        blob_f[:C, 512:640] = in_w[DI:].T
        blob_f[:C, 640:704] = inp["res_w"].T
        blob_f[:, 704] = inp[pre + "conv_b"]
        blob_f[:, 705] = inp[pre + "dt_b"]
        blob_f[:, 706:722] = -np.exp(inp[pre + "A_log"])
        blob_f[:, 722] = inp[pre + "D"]
        blob_f[:C, 723] = inp["conv_b"]
        blob_f[:C, 724] = inp["res_b"]
        blob_f[:C, 725] = inp["bn_gamma"]
        blob_f[:C, 726] = inp["bn_beta"]
        # direction mask: rows of the exchange buffer this core owns
        blob_f[d * C:(d + 1) * C, 727] = 1.0
        blob_h = np.zeros((128, BH_COLS), np.float32)
        blob_h[:, 0:9 * C] = c3
        blob_h[:, 9 * C:9 * C + C] = inp[pre + "out_w"].T
        blob_h[:, 9 * C + C:9 * C + C + 128] = bigproj.T
        blob_h[:, 9 * C + C + 128:9 * C + C + 160] = xproj_w[DTR:].T
        m = {
            "x_loc": np.ascontiguousarray(x_loc),
            "blob_f": blob_f,
            "blob_h": blob_h.astype(ml_dtypes.bfloat16),
        }
        maps.append(m)
    return maps


def _run(inputs, trace=False):
    nc = _get_nc()
    maps = _prep_in_maps(inputs)
    res = bass_utils.run_bass_kernel_spmd(
        nc, maps, core_ids=list(range(NCORE)), trace=trace)
    out = np.stack([res.results[2 * b]["out"].reshape(C, H, W)
                    for b in range(B)])
    return out, res


def kernel(**inputs) -> np.ndarray:
    out, _ = _run(inputs, trace=False)
    return out


# revision 23
# speedup vs baseline: 1.1635x; 1.1635x over previous
"""BiMamba block kernel for 8 Trainium2 NeuronCores.

Sharding: core = 2*sample + direction (4 samples x 2 scan directions).
Each core runs the full mamba for its (sample, direction).

v2 layout: the selective scan dominates on DVE (16 states x 4096 cols at
~2.1 ns/col is irreducible), so the kernel is organized to keep the DVE
queue dense and hide everything else under it:

- Front: fused in-proj + causal depthwise conv as 4 accumulating
  matmuls; true-Silu activations (chunks 0-3 via the silu table before
  any exp work; chunks 4-7 via the tanh identity inside the exp table
  set so no table reload interrupts the first segment's exps).
- Scan: 3 segments (2048/1536/512 cols).  Per state: ACT exp ->
  DVE dbx mul -> DVE tensor_tensor_scan -> DVE C mul -> PE PSUM
  accumulate.  B/C rows arrive via DRAM partition-broadcast DMAs,
  issued a few states ahead.
- Tail: after each segment the out-projection halves are exchanged with
  the pair core via a masked AllReduce (own direction's rows scaled by
  a 0/1 mask from the blob, so the reduce concatenates), giving every
  core the full 128-channel field; the 3x3 conv then runs locally with
  no collective after it.  The reference never un-flips y2, so the odd
  core's natural order already matches the conv row order.  BatchNorm
  stats AllReduce over all 8 cores (each sample counted twice); invstd
  via ln/exp so no extra ACT table load on the critical path.
"""
import os
import sys

for _p in ("/opt/trn_rl_repo", "/root/.axon_site/_ro/trn_rl_repo"):
    if os.path.isdir(_p):
        if _p not in sys.path:
            sys.path.insert(0, _p)
        break

import ml_dtypes
import numpy as np

# The agent image's antenv lacks axon_hooks; inject it so trace=True can
# capture NTFF profiles (used by test.py for HW timing, not for grading).
try:
    import antenv.axon_hooks  # noqa: F401
except ImportError:
    try:
        import types as _types

        from trn_agent_boot.trn_boot import _ntff_profile_via_ctypes

        _hook = _ntff_profile_via_ctypes("/opt/axon/libaxon_pjrt.so")
        _m = _types.ModuleType("antenv.axon_hooks")
        _m.get_axon_ntff_profile_hook = lambda: _hook
        _m.set_axon_ntff_profile_hook = lambda h: None
        sys.modules["antenv.axon_hooks"] = _m
    except Exception:
        pass

import concourse.bass as bass
import concourse.mybir as mybir
from concourse import bacc
from concourse import bass_utils
from concourse.masks import make_identity
from concourse.tile import TileContext

F32 = mybir.dt.float32
BF16 = mybir.dt.bfloat16
AF = mybir.ActivationFunctionType
OP = mybir.AluOpType

B, C, H, W = 4, 64, 64, 64
L = H * W          # 4096
DI = 128           # d_inner
DS = 16            # d_state
DTR = 4            # dt_rank
DCONV = 4
NCORE = 8
CH = 512           # matmul free-dim chunk
NCH = L // CH      # 8
RPC = CH // W      # output rows per chunk (8)

SEGS = ((0, 2048), (2048, 3584), (3584, 4096))
SEG_CHUNKS = ((0, 1, 2, 3), (4, 5, 6), (7,))
NSEG = len(SEGS)

BH_COLS = 9 * C + C + 128 + 32   # c3w | owT | bigT | bcwT
BF_COLS = 736


def _build():
    nc = bacc.Bacc(target_bir_lowering=False, debug=False, num_devices=NCORE)

    def din(name, shape, dtype=F32):
        return nc.dram_tensor(name, shape, dtype, kind="ExternalInput")

    F32R = mybir.dt.float32r
    x_loc = din("x_loc", [C, L], F32R)
    blob_f = din("blob_f", [128, BF_COLS], F32R)
    blob_h = din("blob_h", [128, BH_COLS], BF16)

    out_d = nc.dram_tensor("out", [C, L], F32, kind="ExternalOutput")

    with TileContext(nc) as tc:
        with tc.tile_pool(name="pers", bufs=1) as pers:
            # ---- params arrive as two packed blobs ----
            p_bf = pers.tile([128, BF_COLS], F32R)
            p_bh = pers.tile([128, BH_COLS], BF16)
            nc.sync.dma_start(p_bf[:], blob_f[:])
            nc.sync.dma_start(p_bh[:], blob_h[:])
            p_wk = [p_bf[:, 128 * k:128 * (k + 1)] for k in range(DCONV)]
            p_zwT = p_bf[:, 512:640]
            p_rwT = p_bf[:, 640:704]
            p_c1b = p_bf[:, 704:705].bitcast(F32)
            p_dtb = p_bf[:, 705:706].bitcast(F32)
            p_A = p_bf[:, 706:722].bitcast(F32)
            p_D = p_bf[:, 722:723].bitcast(F32)
            p_c3b = p_bf[:C, 723:724].bitcast(F32)
            p_rb = p_bf[:C, 724:725].bitcast(F32)
            p_bng = p_bf[:C, 725:726].bitcast(F32)
            p_bnb = p_bf[:C, 726:727].bitcast(F32)
            p_mask = p_bf[:, 727:728].bitcast(F32)   # [128,1] direction mask
            p_c1bh = p_bf[:, 728:729].bitcast(F32)   # 0.5*conv1 bias
            p_c3w = p_bh[:, 0:9 * C]
            p_owT = p_bh[:, 9 * C:9 * C + C]
            p_bigT = p_bh[:, 9 * C + C:9 * C + C + 128]
            p_bcwT = p_bh[:, 9 * C + C + 128:9 * C + C + 160]

            ident = pers.tile([128, 128], F32)
            make_identity(nc, ident[:])
            ident_g = pers.tile([128, 128], BF16)
            nc.vector.tensor_copy(ident_g[:], ident[:])

            # DRAM staging for B/C rows (DMA partition-broadcast needs a
            # DRAM source)
            bc_dram = nc.dram_tensor("bc_stage", [2 * DS, L], BF16)

            x_pad = pers.tile([64, 3 + L], F32R)
            nc.gpsimd.memset(x_pad[:, 0:3].bitcast(F32), 0.0)
            # split load so chunk-0 matmuls start early
            nc.sync.dma_start(x_pad[:, 3:3 + 1024], x_loc[:, 0:1024])
            nc.sync.dma_start(x_pad[:, 3 + 1024:3 + L], x_loc[:, 1024:L])

            with tc.tile_pool(name="smid", bufs=1) as smid, \
                 tc.tile_pool(name="ps", bufs=4, space="PSUM") as psp, \
                 tc.tile_pool(name="psy", bufs=4, space="PSUM") as psy, \
                 tc.tile_pool(name="sl_e", bufs=2) as plex, \
                 tc.tile_pool(name="sl_a", bufs=3) as pla, \
                 tc.tile_pool(name="sl_b", bufs=6) as plb, \
                 tc.tile_pool(name="sl_x", bufs=3) as plx, \
                 tc.tile_pool(name="sl_h", bufs=3) as plh, \
                 tc.tile_pool(name="sl_c", bufs=6) as plc, \
                 tc.tile_pool(name="sl_g", bufs=3) as plg, \
                 tc.tile_pool(name="sl_f", bufs=2) as plf, \
                 tc.tile_pool(name="dram", bufs=1, space="DRAM") as dr:
                z_sil = smid.tile([DI, L], BF16)
                dtv = smid.tile([DI, L], BF16)
                dtxc = smid.tile([DI, L], BF16)
                xcd = smid.tile([DI, L], BF16)
                xc = smid.tile([DI, L], BF16)
                carry = smid.tile([DI, DS], F32)

                ympad = smid.tile([128, H + 2, W + 2], BF16)
                nc.gpsimd.memset(ympad[:], 0.0)
                res_sb = smid.tile([C, L], BF16)
                conv_sb = smid.tile([C, L], BF16)
                stats_m = smid.tile([C, NCH], F32)
                stats_v = smid.tile([C, NCH], F32)
                PAIRS = [[0, 1], [2, 3], [4, 5], [6, 7]]

                cc_ins, cc_outs = [], []
                for wi, cvs in enumerate(SEG_CHUNKS):
                    cc_ins.append(dr.tile([C, len(cvs) * CH], BF16,
                                          name=f"cci{wi}"))
                    cc_outs.append(dr.tile([128, len(cvs) * CH], BF16,
                                           name=f"cco{wi}"))
                st_in_a = dr.tile([C, 2], F32, name="st_in_a")
                st_in_b = dr.tile([C, 2], F32, name="st_in_b")
                st_out_a = nc.dram_tensor("st_out_a", [C, 2], F32,
                                          addr_space="Shared")
                st_out_b = nc.dram_tensor("st_out_b", [C, 2], F32,
                                          addr_space="Shared")

                def in_proj(c):
                    """Fused in-proj + causal depthwise conv and gate
                    projection for chunk c; returns the two PSUM tiles."""
                    ps = psp.tile([128, CH], F32, tag="ps")
                    for k in range(DCONV):
                        nc.tensor.matmul(ps[:DI], p_wk[k][:C],
                                         x_pad[:, c * CH + k:c * CH + k + CH],
                                         start=(k == 0), stop=(k == DCONV - 1))
                    ps2 = psp.tile([128, CH], F32, tag="ps")
                    nc.tensor.matmul(ps2[:DI], p_zwT[:C],
                                     x_pad[:, 3 + c * CH:3 + (c + 1) * CH],
                                     start=True, stop=True)
                    return ps, ps2

                def silu_direct(c, ps, ps2):
                    # true Silu from the silu table set (front chunks 0-3)
                    sl = slice(c * CH, (c + 1) * CH)
                    nc.scalar.activation(xc[:, sl], ps[:DI], AF.Silu,
                                         bias=p_c1b[:, 0:1])
                    nc.scalar.activation(z_sil[:, sl], ps2[:DI], AF.Silu)

                def silu_tanh(c, ps, ps2):
                    # silu via 0.5*(v+b)*(1+tanh((v+b)/2)) -- tanh lives in
                    # the exp table set, so chunks 4-7 can run between the
                    # segment-0 exps without a table reload
                    sl = slice(c * CH, (c + 1) * CH)
                    for src, dst, hb in ((ps, xc, p_c1bh), (ps2, z_sil, None)):
                        kw = {"bias": hb[:, 0:1]} if hb is not None else {}
                        th = plf.tile([DI, CH], BF16, tag="th")
                        nc.scalar.activation(th[:], src[:DI], AF.Tanh,
                                             scale=0.5, **kw)
                        raw = plf.tile([DI, CH], BF16, tag="raw")
                        nc.scalar.activation(raw[:], src[:DI], AF.Identity,
                                             scale=0.5, **kw)
                        nc.vector.scalar_tensor_tensor(
                            dst[:, sl], th[:], 1.0, raw[:],
                            op0=OP.add, op1=OP.mult)

                def dt_bc(c):
                    """dt projection (exp part) + B/C projection for chunk
                    c.  Softplus is finished by a batched Ln later."""
                    sl = slice(c * CH, (c + 1) * CH)
                    ps3 = psp.tile([128, CH], F32, tag="ps")
                    nc.tensor.matmul(ps3[:DI], p_bigT[:], xc[:, sl],
                                     start=True, stop=True)
                    nc.scalar.activation(dtv[:, sl], ps3[:DI], AF.Exp,
                                         bias=p_dtb[:, 0:1])
                    ps4 = psp.tile([128, CH], F32, tag="ps")
                    nc.tensor.matmul(ps4[:2 * DS], p_bcwT[:], xc[:, sl],
                                     start=True, stop=True)
                    bch = plb.tile([2 * DS, CH], BF16, tag="bch")
                    nc.scalar.copy(bch[:], ps4[:2 * DS])
                    nc.sync.dma_start(bc_dram[:, sl], bch[:])

                def post_softplus(cs):
                    hsl = slice(cs[0] * CH, (cs[-1] + 1) * CH)
                    nc.scalar.activation(dtv[:, hsl], dtv[:, hsl], AF.Ln,
                                         bias=1.0)
                    for c in cs:
                        sl = slice(c * CH, (c + 1) * CH)
                        nc.vector.tensor_mul(dtxc[:, sl], dtv[:, sl],
                                             xc[:, sl])
                        nc.scalar.activation(xcd[:, sl], xc[:, sl],
                                             AF.Copy, scale=p_D[:, 0:1])

                def seg_end(q):
                    """Out-projection, pair AllGather of the own-direction
                    rows, ympad write and residual for segment q's
                    chunks.  AllGather output is rank-ordered, so both
                    cores receive [dir0 rows; dir1 rows]."""
                    cvs = SEG_CHUNKS[q]
                    stage = plex.tile([C, len(cvs) * CH], BF16,
                                      tag="stage", name=f"stage{q}")
                    for j, cix in enumerate(cvs):
                        sl = slice(cix * CH, (cix + 1) * CH)
                        ssl = slice(j * CH, (j + 1) * CH)
                        yg = plf.tile([DI, CH], BF16, tag="yg")
                        nc.vector.tensor_mul(yg[:], y_ps[cix][:DI],
                                             z_sil[:, sl])
                        po = psp.tile([128, CH], F32, tag="ps",
                                      name=f"po{cix}")
                        nc.tensor.matmul(po[:C], p_owT[:], yg[:],
                                         start=True, stop=True)
                        nc.scalar.copy(stage[:, ssl], po[:C])
                        psr = psp.tile([128, CH], F32, tag="ps",
                                       name=f"rs{cix}")
                        nc.tensor.matmul(psr[:C], p_rwT[:C],
                                         x_pad[:, 3 + cix * CH:
                                               3 + (cix + 1) * CH],
                                         start=True, stop=True)
                        nc.scalar.activation(res_sb[:, sl], psr[:C],
                                             AF.Identity, bias=p_rb[:, 0:1])
                    nc.sync.dma_start(cc_ins[q][:], stage[:])
                    nc.gpsimd.collective_compute(
                        "AllGather", OP.bypass, replica_groups=PAIRS,
                        ins=[cc_ins[q][:].opt()], outs=[cc_outs[q][:].opt()])
                    r0 = cvs[0] * RPC
                    nrows = len(cvs) * RPC
                    nc.sync.dma_start(
                        ympad[:, 1 + r0:1 + r0 + nrows, 1:1 + W],
                        cc_outs[q][:].rearrange("p (r w) -> p r w", w=W))

                def conv3_chunk(c):
                    ps = psp.tile([128, CH], F32, tag="ps", name=f"cv{c}")
                    ps3 = ps[:C].rearrange("p (r w) -> p r w", w=W)
                    r0 = c * RPC
                    n = 0
                    for ky in range(3):
                        for kx in range(3):
                            nc.tensor.matmul(
                                ps3[:],
                                p_c3w[:, (ky * 3 + kx) * C:
                                      (ky * 3 + kx + 1) * C],
                                ympad[:, r0 + ky:r0 + ky + RPC, kx:kx + W],
                                start=(n == 0), stop=(n == 8))
                            n += 1
                    sl = slice(c * CH, (c + 1) * CH)
                    flat = ps3.rearrange("p r w -> p (r w)")
                    nc.scalar.activation(conv_sb[:, sl], flat,
                                         AF.Identity, bias=p_c3b[:, 0:1],
                                         accum_out=stats_m[:, c:c + 1])
                    sq = plf.tile([C, CH], BF16, tag="sq")
                    nc.scalar.activation(sq[:], conv_sb[:, sl],
                                         AF.Square,
                                         accum_out=stats_v[:, c:c + 1])

                stats_a = smid.tile([C, 2], F32)

                def stats_early():
                    nc.vector.tensor_reduce(stats_a[:, 0:1],
                                            stats_m[:, 0:6],
                                            axis=mybir.AxisListType.X,
                                            op=OP.add)
                    nc.vector.tensor_reduce(stats_a[:, 1:2],
                                            stats_v[:, 0:6],
                                            axis=mybir.AxisListType.X,
                                            op=OP.add)
                    nc.sync.dma_start(st_in_a[:], stats_a[:])
                    nc.gpsimd.collective_compute(
                        "AllReduce", OP.add,
                        replica_groups=[[0, 1, 2, 3, 4, 5, 6, 7]],
                        ins=[st_in_a[:].opt()], outs=[st_out_a[:].opt()])

                # =========== emission ===========
                # front chunks 0-3: silu-table silus first
                for c in range(4):
                    ps, ps2 = in_proj(c)
                    silu_direct(c, ps, ps2)
                # nl_exp set: dt exp + softplus + staging for chunks 0-3
                for c in range(4):
                    dt_bc(c)
                post_softplus((0, 1, 2, 3))

                y_ps = {}
                bc_q = {}
                PREF = 5

                def bc_issue(q, s):
                    t0, t1 = SEGS[q]
                    SEG = t1 - t0
                    qsl = slice(t0, t1)
                    bbc = plb.tile([DI, SEG], BF16, tag="bbc",
                                   name=f"bbc{q}_{s}")
                    nc.sync.dma_start(
                        bbc[:],
                        bc_dram[s:s + 1, qsl].to_broadcast((DI, SEG)))
                    cbc = plc.tile([DI, SEG], BF16, tag="cbc",
                                   name=f"cbc{q}_{s}")
                    nc.sync.dma_start(
                        cbc[:],
                        bc_dram[DS + s:DS + s + 1, qsl].to_broadcast(
                            (DI, SEG)))
                    bc_q[(q, s)] = (bbc, cbc)

                def seg_prefetch(q):
                    for s in range(PREF):
                        bc_issue(q, s)

                def seg_scan(q, front_work):
                    t0, t1 = SEGS[q]
                    SEG = t1 - t0
                    qsl = slice(t0, t1)
                    for cix in SEG_CHUNKS[q]:
                        yp = psy.tile([128, CH], F32, tag="yps",
                                      name=f"y{cix}")
                        nc.tensor.matmul(yp[:DI], ident_g[:],
                                         xcd[:, cix * CH:(cix + 1) * CH],
                                         start=True, stop=False)
                        y_ps[cix] = yp
                    for s in range(DS):
                        da = pla.tile([DI, SEG], BF16, tag="da")
                        nc.scalar.activation(da[:], dtv[:, qsl], AF.Exp,
                                             scale=p_A[:, s:s + 1])
                        bbc, cbc = bc_q.pop((q, s))
                        dbx = plx.tile([DI, SEG], BF16, tag="dbx")
                        nc.vector.tensor_mul(dbx[:], dtxc[:, qsl], bbc[:])
                        h = plh.tile([DI, SEG], BF16, tag="h")
                        init = 0.0 if q == 0 else carry[:, s:s + 1]
                        nc.vector.tensor_tensor_scan(h[:], da[:], dbx[:],
                                                     init, op0=OP.mult,
                                                     op1=OP.add)
                        if q < NSEG - 1:
                            # on DVE so the in-order ACT queue of exps is
                            # never blocked behind a scan result
                            nc.vector.tensor_copy(carry[:, s:s + 1],
                                                  h[:, SEG - 1:SEG])
                        g = plg.tile([DI, SEG], BF16, tag="g")
                        nc.vector.tensor_mul(g[:], h[:], cbc[:])
                        for j, cix in enumerate(SEG_CHUNKS[q]):
                            nc.tensor.matmul(
                                y_ps[cix][:DI], ident_g[:],
                                g[:, j * CH:(j + 1) * CH],
                                start=False, stop=(s == DS - 1))
                        if s + PREF < DS:
                            bc_issue(q, s + PREF)
                        if s in front_work:
                            front_work[s]()

                # segment 0 with front chunks 4-7 interleaved between
                # states (tanh-form silu: stays inside the exp table set)
                def fw(c, with_ln):
                    def go():
                        ps, ps2 = in_proj(c)
                        silu_tanh(c, ps, ps2)
                        dt_bc(c)
                        if with_ln:
                            post_softplus((4, 5, 6, 7))
                    return go

                seg_prefetch(0)
                seg_scan(0, {2: fw(4, False), 5: fw(5, False),
                             8: fw(6, False), 11: fw(7, True)})
                seg_prefetch(1)
                seg_end(0)
                seg_scan(1, {8: lambda: conv3_chunk(0),
                             11: lambda: conv3_chunk(1),
                             14: lambda: conv3_chunk(2)})
                seg_prefetch(2)
                seg_end(1)
                seg_scan(2, {})
                seg_end(2)
                # conv 3-5 hide the last exchange's latency on the PE queue
                conv3_chunk(3)
                conv3_chunk(4)
                conv3_chunk(5)
                stats_early()
                conv3_chunk(6)
                conv3_chunk(7)

                # ---- batch stats AllReduces + BN + residual + leaky ----
                # stats split: chunks 0-5 AllReduced while conv 6-7 run,
                # only the tiny chunk 6-7 reduce sits on the tail
                tl = smid
                stot = tl.tile([C, 2], F32)
                stot_b = tl.tile([C, 2], F32)
                stats = tl.tile([C, 2], F32)
                nc.vector.tensor_reduce(stats[:, 0:1], stats_m[:, 6:8],
                                        axis=mybir.AxisListType.X, op=OP.add)
                nc.vector.tensor_reduce(stats[:, 1:2], stats_v[:, 6:8],
                                        axis=mybir.AxisListType.X, op=OP.add)
                nc.sync.dma_start(st_in_b[:], stats[:])
                nc.gpsimd.collective_compute(
                    "AllReduce", OP.add,
                    replica_groups=[[0, 1, 2, 3, 4, 5, 6, 7]],
                    ins=[st_in_b[:].opt()], outs=[st_out_b[:].opt()])
                nc.sync.dma_start(stot[:], st_out_a[:])
                nc.sync.dma_start(stot_b[:], st_out_b[:])
                nc.vector.tensor_add(stot[:], stot[:], stot_b[:])

                # every sample's full conv is present on both pair cores,
                # so the 8-core sum double counts: divide by 2*B*L
                inv = 1.0 / (2.0 * B * L)
                mean = tl.tile([C, 1], F32)
                ex2 = tl.tile([C, 1], F32)
                var = tl.tile([C, 1], F32)
                tmp = tl.tile([C, 1], F32)
                nc.vector.tensor_scalar_mul(mean[:], stot[:, 0:1], inv)
                nc.vector.tensor_scalar_mul(ex2[:], stot[:, 1:2], inv)
                nc.vector.tensor_mul(tmp[:], mean[:], mean[:])
                nc.vector.tensor_sub(var[:], ex2[:], tmp[:])
                # invstd = exp(-0.5*ln(var+eps)) -- ln/exp stay in the
                # loaded table set (no sqrt-set reload on the tail)
                nc.vector.tensor_scalar_add(var[:], var[:], 1e-5)
                nc.scalar.activation(tmp[:], var[:], AF.Ln)
                nc.scalar.activation(tmp[:], tmp[:], AF.Exp, scale=-0.5)
                scal = tl.tile([C, 1], F32)
                shft = tl.tile([C, 1], F32)
                nc.vector.tensor_mul(scal[:], p_bng[:], tmp[:])
                nc.vector.tensor_mul(tmp[:], mean[:], scal[:])
                nc.vector.tensor_sub(shft[:], p_bnb[:], tmp[:])

                # bn + residual + leaky relu: out = prelu(conv*scal + res
                # + shft); conv*scal on ACT (per-partition scale), add on
                # DVE at 2x, prelu+shift on ACT straight to f32 out
                for lo in range(0, L, 1024):
                    hi = lo + 1024
                    bs = plf.tile([C, 1024], BF16, tag="bn")
                    nc.scalar.activation(bs[:], conv_sb[:, lo:hi],
                                         AF.Copy, scale=scal[:, 0:1])
                    nc.vector.tensor_add(bs[:], bs[:], res_sb[:, lo:hi])
                    ot = plf.tile([C, 1024], F32, tag="ot")
                    nc.scalar.activation(ot[:], bs[:],
                                         AF.Prelu, alpha=0.01,
                                         bias=shft[:, 0:1])
                    nc.sync.dma_start(out_d[:, lo:hi], ot[:])

    nc.compile()
    return nc


_NC = None


def _get_nc():
    global _NC
    if _NC is None:
        _NC = _build()
    return _NC


def _prep_in_maps(inp):
    inp = {k: np.asarray(v, dtype=np.float32) for k, v in inp.items()}
    x = inp["x"]  # (4, 64, 64, 64)
    # full 3x3 conv weights over both direction blocks, [in=128, 9*64]
    c3 = np.zeros((128, 9 * C), np.float32)
    for ky in range(3):
        for kx in range(3):
            c3[:, (ky * 3 + kx) * C:(ky * 3 + kx + 1) * C] = \
                inp["conv_w"][:, :, ky, kx].T
    maps = []
    for core in range(NCORE):
        b, d = core // 2, core % 2
        pre = "m1_" if d == 0 else "m2_"
        in_w = inp[pre + "in_w"]          # (256, 64)
        xproj_w = inp[pre + "xproj_w"]    # (36, 128)
        dt_w = inp[pre + "dt_w"]          # (128, 4)
        conv1_w = inp[pre + "conv_w"]     # (128, 4)

        x_loc = x[b].reshape(C, L)
        if d == 1:
            x_loc = x_loc[:, ::-1]

        bigproj = dt_w @ xproj_w[:DTR]    # (128, 128)

        blob_f = np.zeros((128, BF_COLS), np.float32)
        # fused in-projection + depthwise causal conv:
        # W_k[ch_x, di] = in_w[di, ch_x] * conv1_w[di, k]
        xi_w = in_w[:DI]                  # (128, 64)
        for k in range(DCONV):
            blob_f[:C, 128 * k:128 * (k + 1)] = \
                (xi_w * conv1_w[:, k:k + 1]).T
        blob_f[:C, 512:640] = in_w[DI:].T
        blob_f[:C, 640:704] = inp["res_w"].T
        blob_f[:, 704] = inp[pre + "conv_b"]
        blob_f[:, 705] = inp[pre + "dt_b"]
        blob_f[:, 706:722] = -np.exp(inp[pre + "A_log"])
        blob_f[:, 722] = inp[pre + "D"]
        blob_f[:C, 723] = inp["conv_b"]
        blob_f[:C, 724] = inp["res_b"]
        blob_f[:C, 725] = inp["bn_gamma"]
        blob_f[:C, 726] = inp["bn_beta"]
        # direction mask: rows of the exchange buffer this core owns
        blob_f[d * C:(d + 1) * C, 727] = 1.0
        blob_h = np.zeros((128, BH_COLS), np.float32)
        blob_h[:, 0:9 * C] = c3
        blob_h[:, 9 * C:9 * C + C] = inp[pre + "out_w"].T
        blob_h[:, 9 * C + C:9 * C + C + 128] = bigproj.T
        blob_h[:, 9 * C + C + 128:9 * C + C + 160] = xproj_w[DTR:].T
        m = {
            "x_loc": np.ascontiguousarray(x_loc),
            "blob_f": blob_f,
            "blob_h": blob_h.astype(ml_dtypes.bfloat16),
        }
        maps.append(m)
    return maps


def _run(inputs, trace=False):
    nc = _get_nc()
    maps = _prep_in_maps(inputs)
    res = bass_utils.run_bass_kernel_spmd(
        nc, maps, core_ids=list(range(NCORE)), trace=trace)
    out = np.stack([res.results[2 * b]["out"].reshape(C, H, W)
                    for b in range(B)])
    return out, res


def kernel(**inputs) -> np.ndarray:
    out, _ = _run(inputs, trace=False)
    return out
